# revision 13
# baseline (speedup 1.0000x reference)
"""Mistral decoder layer (B=1, S=1024, HID=4096, 32 heads, INTER=11008), fp32 in/out,
tensor-parallel over 8 trn2 NeuronCores (Megatron style).

Everything on-chip runs in the "transposed" domain ([feature, seq] layouts) so
no on-chip transposes are needed anywhere:
  - host pre-transposes x, the mask, and all weight shards (and folds the RMS
    norm gains + 1/sqrt(hd) scale into the weights); weights/activations are
    fp16 on device (PSUM accumulation stays fp32)
  - RMSNorm per-token sums-of-squares are computed with ones-matmuls on the
    TensorE (reduction over the partition dim), which also broadcasts the
    result to all 128 partitions for free
  - attention runs in fp32/fp32r: scores are computed transposed (ST[t, q]),
    softmax-exp applied elementwise, the PV product contracts over t with
    expST as lhsT, producing attnT[hd, q] directly; rowsums come from
    ones-matmuls and the 1/rowsum normalization is applied at the evacuation
  - residuals are folded into the collectives: each core contributes
    partial + residual/8, so the collective output is already residual-added
  - both collectives are chunked along the SEQUENCE dim so the consumer's
    column-chunk 0 compute overlaps the chunk-1 collective:
      o_proj  -> AllReduce per seq-chunk (MLP needs the full hidden dim)
      down_proj -> ReduceScatter per seq-chunk (each core ends with a 512-row
      shard of the output; the host concatenates the 8 shards)
"""

import numpy as np

import concourse.bacc as bacc
import concourse.mybir as mybir
import concourse.tile as tile
from concourse.bass_utils import run_bass_kernel_spmd

AF = mybir.ActivationFunctionType
ALU = mybir.AluOpType
F32 = mybir.dt.float32
F32R = mybir.dt.float32r
F16 = mybir.dt.float16

N_CORES = 8
HID = 4096
S = 1024
NH = 32
HD = 128
NH_L = NH // N_CORES          # 4 local heads
DL = NH_L * HD                # 512 local q/k/v dims
INTER = 11008
IL_T = 11                     # local intermediate k-tiles (padded)
IL = IL_T * 128               # 1408 padded local intermediate
ILR = INTER // N_CORES        # 1376 real local intermediate
KT = HID // 128               # 32 hidden k-tiles
CH = 2                        # seq chunks (also the collective chunks)
CW = S // CH                  # 512
TB = S // 128                 # 8 seq tiles of 128
OSH = HID // N_CORES          # 512 output rows per core (ReduceScatter shard)
EPS = 1e-5

_CACHE = {}


def _r(ap):
    return ap.bitcast(F32R)


def _build(collectives=True, repeat=1):
    nc = bacc.Bacc("TRN2", target_bir_lowering=False, debug=False,
                   num_devices=N_CORES)

    xT = nc.dram_tensor("xT", [HID, S], F16, kind="ExternalInput").ap()
    maskTd = nc.dram_tensor("maskTd", [TB, 128, 128], F32, kind="ExternalInput").ap()
    wqT = nc.dram_tensor("wqT", [HID, DL], F16, kind="ExternalInput").ap()
    wkT = nc.dram_tensor("wkT", [HID, DL], F16, kind="ExternalInput").ap()
    wvT = nc.dram_tensor("wvT", [HID, DL], F16, kind="ExternalInput").ap()
    woD = nc.dram_tensor("woD", [NH_L, 128, HID], F16, kind="ExternalInput").ap()
    wuD = nc.dram_tensor("wuD", [IL_T, 128, HID], F16, kind="ExternalInput").ap()
    wgD = nc.dram_tensor("wgD", [IL_T, 128, HID], F16, kind="ExternalInput").ap()
    wdD = nc.dram_tensor("wdD", [KT, 128, IL], F16, kind="ExternalInput").ap()
    outT = nc.dram_tensor("outT", [OSH, S], F16, kind="ExternalOutput").ap()

    ob = [nc.dram_tensor(f"ob{c}", [HID, CW], F16).ap() for c in range(CH)]
    h2d = [nc.dram_tensor(f"h2d{c}", [HID, CW], F16, addr_space="Shared").ap()
           for c in range(CH)]
    s1_d = nc.dram_tensor("s1_d", [S], F32).ap()
    dnb = [nc.dram_tensor(f"dnb{c}", [HID, CW], F16).ap() for c in range(CH)]
    dnr = [nc.dram_tensor(f"dnr{c}", [OSH, CW], F16).ap()
           for c in range(CH)]

    rg = [list(range(N_CORES))]

    def all_reduce(dst, srcs, engine):
        if collectives:
            engine.collective_compute(
                "AllReduce", ALU.add, ins=[srcs[:]], outs=[dst[:]],
                replica_groups=rg)
        else:
            engine.dma_start(dst[:], srcs[:])

    def reduce_scatter(dst, srcs, engine):
        if collectives:
            engine.collective_compute(
                "ReduceScatter", ALU.add, ins=[srcs[:]], outs=[dst[:]],
                replica_groups=rg)
        else:
            engine.dma_start(dst[:], srcs[0:OSH, :])

    with tile.TileContext(nc) as tc:
      for rep in range(repeat):
        P = f"r{rep}_" if repeat > 1 else ""
        with tc.tile_pool(name=P + "const", bufs=1) as const:
            ones16 = const.tile([128, 128], F16, tag="ones16")
            nc.vector.memset(ones16[:], 1.0)
            s1 = const.tile([128, S], F32, tag="s1")
            s1t = const.tile([128, TB], F32, tag="s1t")
            epst = const.tile([128, 1], F32, tag="epst")
            nc.vector.memset(epst[:], EPS)

            # ======== Phases 0-2: x load + RMSNorm stats + QKV (x resident) ====
            with tc.tile_pool(name=P + "qkvo", bufs=1) as qkvo:
                QTt = [qkvo.tile([128, S], F16, tag=f"QT{h}", name=f"QT{h}")
                       for h in range(NH_L)]
                KTt = [qkvo.tile([128, S], F16, tag=f"KT{h}", name=f"KT{h}")
                       for h in range(NH_L)]
                Vt = [qkvo.tile([128, DL], F16, tag=f"V{t}", name=f"V{t}")
                      for t in range(TB)]
                ATt = [qkvo.tile([128, S], F16, tag=f"AT{h}", name=f"AT{h}")
                       for h in range(NH_L)]

                with tc.tile_pool(name=P + "xres", bufs=1) as xres:
                    xt = [xres.tile([128, S], F16, tag=f"x{k}", name=f"x{k}")
                          for k in range(KT)]
                    with (
                        tc.tile_pool(name=P + "p0", bufs=2) as p0,
                        tc.tile_pool(name=P + "p0m", bufs=2) as p0m,
                        tc.tile_pool(name=P + "p0ps", bufs=1, space="PSUM") as p0ps,
                    ):
                        r2 = [p0ps.tile([128, CW], F32, tag=f"r2_{c}",
                                        name=f"r2_{c}") for c in range(CH)]
                        for k in range(KT):
                            nc.sync.dma_start(xt[k][:],
                                              xT[k * 128:(k + 1) * 128, :])
                            sq = p0.tile([128, S], F16, tag="sq", name=f"sq{k}")
                            nc.scalar.activation(sq[:], xt[k][:], AF.Square)
                            for c in range(CH):
                                nc.tensor.matmul(
                                    r2[c][:], ones16[:],
                                    sq[:, c * CW:(c + 1) * CW],
                                    start=(k == 0), stop=(k == KT - 1))
                        for c in range(CH):
                            ms = p0m.tile([128, CW], F32, tag="ms")
                            nc.scalar.activation(ms[:], r2[c][:], AF.Sqrt,
                                                 bias=epst[:], scale=1.0 / HID)
                            nc.vector.reciprocal(s1[:, c * CW:(c + 1) * CW],
                                                 ms[:])
                    # s1t = s1 transposed down partitions, via a DRAM bounce
                    nc.sync.dma_start(s1_d.rearrange("(o s) -> o s", o=1),
                                      s1[0:1, :])
                    nc.sync.dma_start(s1t[:], s1_d.rearrange("(t p) -> p t", p=128))

                    # q/k passes: weights stream, 8 psum groups, evac scales by s1
                    for nm, wT, outs in (("q", wqT, QTt), ("k", wkT, KTt)):
                        with (
                            tc.tile_pool(name=P + f"{nm}w", bufs=3) as wp,
                            tc.tile_pool(name=P + f"{nm}ps", bufs=1,
                                         space="PSUM") as ps,
                        ):
                            pt = [ps.tile([128, CW], F32, tag=f"pt{j}",
                                          name=f"pt{j}") for j in range(NH_L * CH)]
                            for k in range(KT):
                                wt = wp.tile([128, DL], F16, tag="wt")
                                nc.sync.dma_start(
                                    wt[:], wT[k * 128:(k + 1) * 128, :])
                                for h in range(NH_L):
                                    for c in range(CH):
                                        nc.tensor.matmul(
                                            pt[h * CH + c][:],
                                            wt[:, h * 128:(h + 1) * 128],
                                            xt[k][:, c * CW:(c + 1) * CW],
                                            start=(k == 0), stop=(k == KT - 1))
                            for h in range(NH_L):
                                for c in range(CH):
                                    nc.vector.tensor_mul(
                                        outs[h][:, c * CW:(c + 1) * CW],
                                        pt[h * CH + c][:],
                                        s1[:, c * CW:(c + 1) * CW])

                    # v pass: V[t] rows scaled by s1t column
                    with (
                        tc.tile_pool(name=P + "vw", bufs=3) as wp,
                        tc.tile_pool(name=P + "vps", bufs=1, space="PSUM") as ps,
                    ):
                        pt = [ps.tile([128, DL], F32, tag=f"pt{t}", name=f"pt{t}")
                              for t in range(TB)]
                        for k in range(KT):
                            wt = wp.tile([128, DL], F16, tag="wt")
                            nc.sync.dma_start(
                                wt[:], wvT[k * 128:(k + 1) * 128, :])
                            for t in range(TB):
                                nc.tensor.matmul(
                                    pt[t][:], xt[k][:, t * 128:(t + 1) * 128],
                                    wt[:], start=(k == 0), stop=(k == KT - 1))
                        for t in range(TB):
                            nc.vector.tensor_scalar(
                                Vt[t][:], pt[t][:], s1t[:, t:t + 1], None,
                                op0=ALU.mult)

                    # ======== Phase 3+4: attention + o-proj, seq-chunked ======
                    with (
                        tc.tile_pool(name=P + "mask", bufs=1) as mp,
                        tc.tile_pool(name=P + "est", bufs=2) as estp,
                        tc.tile_pool(name=P + "rin", bufs=2) as rinp,
                        tc.tile_pool(name=P + "aps", bufs=1, space="PSUM") as aps,
                        tc.tile_pool(name=P + "stps", bufs=2, space="PSUM") as stps,
                        tc.tile_pool(name=P + "ow", bufs=1) as owp,
                        tc.tile_pool(name=P + "ops", bufs=2, space="PSUM") as ops,
                        tc.tile_pool(name=P + "oev", bufs=3) as oev,
                    ):
                        mtiles = []
                        for t in range(TB):
                            mt = mp.tile([128, 128], F32, tag=f"m{t}", name=f"mk{t}")
                            nc.sync.dma_start(mt[:], maskTd[t, :, :])
                            mtiles.append(mt)
                        wo_sl = []
                        for h in range(NH_L):
                            wt = owp.tile([128, HID], F16, tag=f"wo{h}",
                                          name=f"wo{h}")
                            nc.sync.dma_start(wt[:], woD[h, :, :])
                            wo_sl.append(wt)
                        atp = [aps.tile([128, CW], F32, tag=f"atp{j}",
                                        name=f"atp{j}") for j in range(2)]
                        rsp = [aps.tile([128, CW], F32, tag=f"rsp{j}",
                                        name=f"rsp{j}") for j in range(2)]
                        for c in range(CH):
                            # --- attention for seq chunk c (causal-aware):
                            # tile t covers keys [t*128,(t+1)*128); within
                            # chunk c only query cols >= q0=(t-4c)*128 can
                            # attend to them. The mask add is only needed on
                            # the 128-wide block-diagonal; columns below q0
                            # are skipped entirely (zero contribution).
                            for h in range(NH_L):
                                tbs = list(range(0, (c + 1) * 4))
                                ets = []
                                for t in tbs:
                                    q0 = max(0, (t - 4 * c) * 128)
                                    qs = slice(c * CW + q0, (c + 1) * CW)
                                    stp = stps.tile([128, CW], F32, tag="st")
                                    nc.tensor.matmul(
                                        stp[:, q0:], KTt[h][:, t * 128:(t + 1) * 128],
                                        QTt[h][:, qs], start=True, stop=True)
                                    et = estp.tile([128, CW], F16, tag=f"et{t}",
                                                   name=f"et{t}")
                                    if t >= c * 4:
                                        nc.vector.tensor_add(
                                            et[:, q0:q0 + 128],
                                            stp[:, q0:q0 + 128], mtiles[t][:])
                                        nc.scalar.activation(
                                            et[:, q0:q0 + 128],
                                            et[:, q0:q0 + 128], AF.Exp)
                                        if q0 + 128 < CW:
                                            nc.scalar.activation(
                                                et[:, q0 + 128:],
                                                stp[:, q0 + 128:], AF.Exp)
                                    else:
                                        nc.scalar.activation(et[:], stp[:],
                                                             AF.Exp)
                                    ets.append((et, q0))
                                ap_, rp_ = atp[h % 2], rsp[h % 2]
                                for j, t in enumerate(tbs):
                                    et, q0 = ets[j]
                                    st_, sp_ = (j == 0), (j == len(tbs) - 1)
                                    nc.tensor.matmul(
                                        ap_[:, q0:], Vt[t][:, h * 128:(h + 1) * 128],
                                        et[:, q0:], start=st_, stop=sp_)
                                    nc.tensor.matmul(
                                        rp_[:, q0:], ones16[:], et[:, q0:],
                                        start=st_, stop=sp_)
                                ri = rinp.tile([128, CW], F32, tag="ri")
                                nc.vector.reciprocal(ri[:], rp_[:])
                                nc.vector.tensor_mul(
                                    ATt[h][:, c * CW:(c + 1) * CW], ap_[:], ri[:])

                            # --- o-proj for seq chunk c + x/8 fold -> AllReduce
                            for mh in range(KT):
                                pt = ops.tile([128, CW], F32, tag="pt")
                                for h in range(NH_L):
                                    nc.tensor.matmul(
                                        pt[:],
                                        wo_sl[h][:, mh * 128:(mh + 1) * 128],
                                        ATt[h][:, c * CW:(c + 1) * CW],
                                        start=(h == 0), stop=(h == NH_L - 1))
                                ev = oev.tile([128, CW], F16, tag="ev")
                                nc.vector.scalar_tensor_tensor(
                                    ev[:], xt[mh][:, c * CW:(c + 1) * CW],
                                    1.0 / N_CORES, pt[:], op0=ALU.mult,
                                    op1=ALU.add)
                                nc.sync.dma_start(
                                    ob[c][mh * 128:(mh + 1) * 128, :], ev[:])
                            all_reduce(h2d[c], ob[c], nc.gpsimd)

            # ===== Phases 5+6: RMSNorm #2 stats + up/gate, seq-chunk-major ====
            # Chunk c's stats + MLP matmuls depend only on AllReduce chunk c,
            # so the whole c=0 sweep overlaps the c=1 AllReduce. s2 is applied
            # at the up/gate PSUM evacuations, so the MLP matmuls consume raw
            # h2 and can start before s2 is ready. h2 arrives as 4 batched
            # 1MB DMAs per chunk on the gpsimd queue so collective-waits
            # never block the sync queue's weight streaming. The d=0 up/gate
            # matmuls are emitted BEFORE the stats chain: they depend only on
            # the h2 DMAs (not on the ScalarE squares), keeping PE dense.
            with tc.tile_pool(name=P + "h2res", bufs=1) as h2p:
                h2g = [[h2p.tile([128, (KT // 4) * CW], F16, tag=f"h2_{c}_{g}",
                                 name=f"h2_{c}_{g}") for g in range(4)]
                       for c in range(CH)]
                s2 = h2p.tile([128, S], F32, tag="s2", name="s2")

                def h2s(k, c):
                    return h2g[c][k // 8][:, (k % 8) * CW:(k % 8 + 1) * CW]

                with tc.tile_pool(name=P + "mres", bufs=1) as mres:
                    m_t = [mres.tile([128, S], F16, tag=f"m{i}", name=f"mres{i}")
                           for i in range(IL_T)]
                    with (
                        tc.tile_pool(name=P + "p5", bufs=3) as p5,
                        tc.tile_pool(name=P + "p5m", bufs=2) as p5m,
                        tc.tile_pool(name=P + "p5ps", bufs=1, space="PSUM") as p5ps,
                        tc.tile_pool(name=P + "ugw", bufs=2) as ugw,
                        tc.tile_pool(name=P + "ugps", bufs=2, space="PSUM") as ugps,
                        tc.tile_pool(name=P + "ugt", bufs=3) as ugt,
                    ):
                        r2 = [p5ps.tile([128, CW], F32, tag=f"r2_{c}",
                                        name=f"r2b_{c}") for c in range(CH)]

                        def ug_mms(c, d):
                            pts = {}
                            for nm, wD in (("u", wuD), ("g", wgD)):
                                sl = ugw.tile([128, HID], F16, tag=nm,
                                              name=f"slab_{nm}{c}_{d}")
                                nc.sync.dma_start(sl[:], wD[d, :, :])
                                pt = ugps.tile([128, CW], F32, tag=f"pt{nm}",
                                               name=f"pt{nm}{c}_{d}")
                                for k in range(KT):
                                    nc.tensor.matmul(
                                        pt[:], sl[:, k * 128:(k + 1) * 128],
                                        h2s(k, c),
                                        start=(k == 0), stop=(k == KT - 1))
                                pts[nm] = pt
                            return pts

                        def ug_evac(c, d, pts):
                            s2c = s2[:, c * CW:(c + 1) * CW]
                            un = ugt.tile([128, CW], F32, tag="un")
                            nc.vector.tensor_mul(un[:], pts["u"][:], s2c)
                            sil = ugt.tile([128, CW], F32, tag="sil")
                            nc.scalar.activation(sil[:], un[:], AF.Silu)
                            gn = ugt.tile([128, CW], F32, tag="gn")
                            nc.vector.tensor_mul(gn[:], pts["g"][:], s2c)
                            nc.vector.tensor_mul(
                                m_t[d][:, c * CW:(c + 1) * CW], sil[:], gn[:])

                        for c in range(CH):
                            for g in range(4):
                                nc.gpsimd.dma_start(
                                    h2g[c][g][:].rearrange("p (k q) -> p k q",
                                                           k=KT // 4),
                                    h2d[c][g * (KT // 4) * 128:
                                           (g + 1) * (KT // 4) * 128, :]
                                    .rearrange("(k p) q -> p k q", p=128))
                            # d=0 matmuls first, then stats, then d=0 evac
                            pts0 = ug_mms(c, 0)
                            for k in range(KT):
                                sq = p5.tile([128, CW], F16, tag="sq")
                                nc.scalar.activation(sq[:], h2s(k, c),
                                                     AF.Square)
                                nc.tensor.matmul(r2[c][:], ones16[:], sq[:],
                                                 start=(k == 0),
                                                 stop=(k == KT - 1))
                            ms = p5m.tile([128, CW], F32, tag="ms")
                            nc.scalar.activation(ms[:], r2[c][:], AF.Sqrt,
                                                 bias=epst[:], scale=1.0 / HID)
                            nc.vector.reciprocal(s2[:, c * CW:(c + 1) * CW],
                                                 ms[:])
                            ug_evac(c, 0, pts0)
                            for d in range(1, IL_T):
                                ug_evac(c, d, ug_mms(c, d))

                    # ===== Phase 7: down-proj + h2/8 fold -> ReduceScatter ====
                    with (
                        tc.tile_pool(name=P + "dw", bufs=3) as dwp,
                        tc.tile_pool(name=P + "dps", bufs=6, space="PSUM") as dps,
                        tc.tile_pool(name=P + "dev", bufs=3) as dev,
                    ):
                        for c in range(CH):
                            for mh in range(KT):
                                sl = dwp.tile([128, IL], F16, tag="dw")
                                nc.sync.dma_start(sl[:], wdD[mh, :, :])
                                pt = dps.tile([128, CW], F32, tag="pt")
                                for i in range(IL_T):
                                    nc.tensor.matmul(
                                        pt[:], sl[:, i * 128:(i + 1) * 128],
                                        m_t[i][:, c * CW:(c + 1) * CW],
                                        start=(i == 0), stop=(i == IL_T - 1))
                                ev = dev.tile([128, CW], F16, tag="ev")
                                nc.vector.scalar_tensor_tensor(
                                    ev[:], h2s(mh, c),
                                    1.0 / N_CORES, pt[:], op0=ALU.mult,
                                    op1=ALU.add)
                                nc.sync.dma_start(
                                    dnb[c][mh * 128:(mh + 1) * 128, :], ev[:])
                            reduce_scatter(dnr[c], dnb[c], nc.gpsimd)
                            # outT write on the gpsimd queue: its wait on the
                            # ReduceScatter must not block sync-queue weight
                            # streaming for the next chunk's down-proj
                            nc.gpsimd.dma_start(outT[:, c * CW:(c + 1) * CW],
                                                dnr[c][:])

    nc.compile()
    return nc


def _host_shard(hidden_states, mask, wq, wk, wv, wo, w_gate, w_up, w_down,
                g_in, g_post):
    x = np.asarray(hidden_states, dtype=np.float32).reshape(S, HID)
    xT = np.ascontiguousarray(x.T).astype(np.float16)
    maskT = np.ascontiguousarray(np.asarray(mask, dtype=np.float32)
                                 .reshape(S, S).T)
    # only the 128x128 block-diagonal of the (transposed) causal mask is
    # nontrivial; off-diagonal blocks are all-0 or all--1e9 and are handled
    # structurally (full tiles / skipped regions)
    maskTd = np.empty((TB, 128, 128), np.float32)
    for t in range(TB):
        maskTd[t] = maskT[t * 128:(t + 1) * 128, t * 128:(t + 1) * 128]
    g_in = np.asarray(g_in, dtype=np.float32)
    g_post = np.asarray(g_post, dtype=np.float32)
    sc = np.float32(HD ** -0.5)

    wq = np.asarray(wq, np.float32)
    wk = np.asarray(wk, np.float32)
    wv = np.asarray(wv, np.float32)
    wo = np.asarray(wo, np.float32)
    w_up = np.asarray(w_up, np.float32)
    w_gate = np.asarray(w_gate, np.float32)
    w_down = np.asarray(w_down, np.float32)

    in_maps = []
    for i in range(N_CORES):
        r0, r1 = i * DL, (i + 1) * DL
        i0, i1 = i * ILR, (i + 1) * ILR
        wqT = (wq[r0:r1].T * (g_in[:, None] * sc)).astype(np.float16)
        wkT = (wk[r0:r1].T * g_in[:, None]).astype(np.float16)
        wvT = (wv[r0:r1].T * g_in[:, None]).astype(np.float16)
        # woD[h, p, m] = wo[m, r0 + h*128 + p]
        woD = np.ascontiguousarray(
            wo[:, r0:r1].T.reshape(NH_L, 128, HID)).astype(np.float16)
        # wuD[d, p, k*128+m] = w_up[i0 + d*128 + m, k*128 + p] * g_post[k*128+p]
        wu = np.zeros((IL, HID), np.float32)
        wu[:ILR] = w_up[i0:i1] * g_post[None, :]
        wuD = np.ascontiguousarray(
            wu.reshape(IL_T, 128, KT, 128).transpose(0, 3, 2, 1)
            .reshape(IL_T, 128, HID)).astype(np.float16)
        wg = np.zeros((IL, HID), np.float32)
        wg[:ILR] = w_gate[i0:i1] * g_post[None, :]
        wgD = np.ascontiguousarray(
            wg.reshape(IL_T, 128, KT, 128).transpose(0, 3, 2, 1)
            .reshape(IL_T, 128, HID)).astype(np.float16)
        # wdD[mh, p, i*128+m] = w_down[mh*128 + m, i0 + i*128 + p]
        wd = np.zeros((HID, IL), np.float32)
        wd[:, :ILR] = w_down[:, i0:i1]
        wdD = np.ascontiguousarray(
            wd.reshape(KT, 128, IL_T, 128).transpose(0, 3, 2, 1)
            .reshape(KT, 128, IL)).astype(np.float16)
        in_maps.append({
            "xT": xT, "maskTd": maskTd, "wqT": wqT, "wkT": wkT, "wvT": wvT,
            "woD": woD, "wuD": wuD, "wgD": wgD, "wdD": wdD,
        })
    return in_maps


def _assemble(outT_stack):
    """[N_CORES, OSH, S] fp16 ReduceScatter shards -> [1, S, HID] fp32."""
    full = np.concatenate([np.asarray(outT_stack[i]) for i in range(N_CORES)],
                          axis=0).astype(np.float32)   # [HID, S]
    return np.ascontiguousarray(full.T).reshape(1, S, HID)


def _get_nc(repeat=1):
    key = ("nc", repeat)
    if key not in _CACHE:
        _CACHE[key] = _build(repeat=repeat)
    return _CACHE[key]


def kernel(**inputs):
    nc = _get_nc()
    in_maps = _host_shard(**{k: np.asarray(v) for k, v in inputs.items()})
    res = run_bass_kernel_spmd(nc, in_maps, list(range(N_CORES)))
    return _assemble([res.results[i]["outT"] for i in range(N_CORES)])


def _make_runner(repeat=1, **inputs):
    """Build the compiled sharded callable + device-resident inputs once.
    Returns run() -> (wall_ns, outs)."""
    import time
    import jax
    from jax.sharding import Mesh, PartitionSpec
    from jax.experimental.shard_map import shard_map
    from concourse import bass2jax

    nc = _get_nc(repeat)
    in_maps = _host_shard(**{k: np.asarray(v) for k, v in inputs.items()})
    bass2jax.install_neuronx_cc_hook()

    partition_name = (nc.partition_id_tensor.name
                      if nc.partition_id_tensor else None)
    in_names, out_names, out_avals, zero_outs = [], [], [], []
    for alloc in nc.m.functions[0].allocations:
        if not isinstance(alloc, mybir.MemoryLocationSet):
            continue
        name = alloc.memorylocations[0].name
        if alloc.kind == "ExternalInput":
            if name != partition_name:
                in_names.append(name)
        elif alloc.kind == "ExternalOutput":
            out_names.append(name)
            shape = tuple(alloc.tensor_shape)
            dtype = mybir.dt.np(alloc.dtype)
            out_avals.append(jax.core.ShapedArray(shape, dtype))
            zero_outs.append(np.zeros(shape, dtype))
    n_params = len(in_names)
    all_in = list(in_names) + list(out_names)
    if partition_name is not None:
        all_in.append(partition_name)

    def _body(*args):
        operands = list(args)
        if partition_name is not None:
            operands.append(bass2jax.partition_id_tensor())
        outs = bass2jax._bass_exec_p.bind(
            *operands,
            out_avals=tuple(out_avals), in_names=tuple(all_in),
            out_names=tuple(out_names), lowering_input_output_aliases=(),
            sim_require_finite=True, sim_require_nnan=True, nc=nc)
        return tuple(outs)

    devices = jax.devices()[:N_CORES]
    mesh = Mesh(np.asarray(devices), ("core",))
    n_outs = len(out_names)
    in_specs = (PartitionSpec("core"),) * (n_params + n_outs)
    out_specs = (PartitionSpec("core"),) * n_outs
    fn = jax.jit(shard_map(_body, mesh=mesh, in_specs=in_specs,
                           out_specs=out_specs, check_rep=False))
    concat_in = [np.concatenate([np.asarray(in_maps[c][nm])
                                 for c in range(N_CORES)], axis=0)
                 for nm in in_names]
    concat_zeros = [np.zeros((N_CORES * z.shape[0], *z.shape[1:]), z.dtype)
                    for z in zero_outs]
    sharding = jax.sharding.NamedSharding(mesh, PartitionSpec("core"))
    dev_in = [jax.device_put(a, sharding) for a in concat_in]
    dev_zero = [jax.device_put(a, sharding) for a in concat_zeros]

    outs = fn(*dev_in, *dev_zero)          # warm-up / compile
    jax.block_until_ready(outs)

    def run():
        t0 = time.perf_counter_ns()
        o = fn(*dev_in, *dev_zero)
        jax.block_until_ready(o)
        return time.perf_counter_ns() - t0, o

    def unpack(o):
        return {nm: np.asarray(o[i]).reshape(N_CORES, *out_avals[i].shape)
                for i, nm in enumerate(out_names)}

    return run, unpack


def bench(iters=8, repeat=1, **inputs):
    """Time repeated on-device executions; returns (best_ns, outputs)."""
    run, unpack = _make_runner(repeat=repeat, **inputs)
    best, outs = float("inf"), None
    for _ in range(iters):
        ns, outs = run()
        best = min(best, ns)
    return best, unpack(outs)
